# revision 14
# baseline (speedup 1.0000x reference)
"""Trainium2 Bass kernel: per-batch segment-mean pooling + 3-layer MLP.

Reference computation (B=64, T=512, H=768, S=128):
  pooled[b,s,:] = mean over t of hidden[b,t,:] where statements_ids[b,t]==s
  x = gelu(pooled @ w1 + b1); x = gelu(x @ w2 + b2)
  out[b,s] = sigmoid(x @ w3 + b3)

Distribution: data-parallel over batch across 8 NeuronCores (8 batches per
core); MLP weights replicated.

v2 design notes (PE streams at N cols/cycle regardless of dtype, so the
wins are bandwidth + gap-closing, not matmul cycles):
  - All tensor streams in bf16: halves HBM traffic (9.75 MB/core vs 17.3)
    so the pooling phase is no longer DMA-paced.
  - The one-hot matrix MT[t,s] (exact 0/1 values in bf16) is built on the
    host and DMA'd in: removes the iota/is_equal/cast DVE chain from the
    critical path at startup.
  - counts -> inv = 1/max(counts,1) computed on host (f32, exact): kills
    the padded ones-columns, the counts matmul and the max/reciprocal
    chain; pooling psum is evacuated with a single f32-psum * inv
    broadcast multiply (DVE) per chunk, writing bf16.
  - All 8 hidden batches are SBUF-resident (bf16 halves footprint) so
    every DMA is issued up front and streams at full rate.
  - Sigmoids are deferred and batched (2 ACT table switches instead of 8;
    each 1.28us): rc0-2 sigmoid mid-run under pool(6/7), only rc3's
    sigmoid (+4KB out DMA) sits on the tail behind the last gelu.
"""

import os
import sys

sys.path.insert(0, "/opt/trn_rl_repo")

import ml_dtypes
import numpy as np

import concourse.bass as bass
import concourse.mybir as mybir
import concourse.tile as tile
from concourse import bacc, bass_utils

B, T, H, S = 64, 512, 768, 128
N_CORES = 8
BL = B // N_CORES  # local batches per core
P = 128
KT = T // P        # t-tiles per batch
KH = H // P        # h-tiles
R = BL * S         # MLP rows per core
RC = 2 * S         # moving-dim chunk (2 batches)
NRC = R // RC
MTC = BL * KT * S  # packed one-hot columns
CH_COLS = P + KH           # bf16 packed consts: ident | w3
CF_COLS = BL + 2 * KH + 1  # f32 packed consts: inv | b1 | b2 | b3

BF16 = ml_dtypes.bfloat16

_CACHE: dict = {}


def _build_program():
    f32, bf16 = mybir.dt.float32, mybir.dt.bfloat16
    FT = mybir.ActivationFunctionType
    OP = mybir.AluOpType

    nc = bacc.Bacc("TRN2", target_bir_lowering=False, debug=False)
    hid = nc.dram_tensor("hidden", [BL, T, H], bf16, kind="ExternalInput").ap()
    mtn = nc.dram_tensor("mtn", [P, MTC], bf16, kind="ExternalInput").ap()
    w1 = nc.dram_tensor("w1", [H, H], bf16, kind="ExternalInput").ap()
    w2 = nc.dram_tensor("w2", [H, H], bf16, kind="ExternalInput").ap()
    cpack_h = nc.dram_tensor("cpack_h", [P, CH_COLS], bf16, kind="ExternalInput").ap()
    cpack_f = nc.dram_tensor("cpack_f", [P, CF_COLS], f32, kind="ExternalInput").ap()
    out = nc.dram_tensor("out", [BL, S], f32, kind="ExternalOutput").ap()

    with tile.TileContext(nc) as tc:
        with (
            tc.tile_pool(name="consts", bufs=1) as consts,
            tc.tile_pool(name="wpool", bufs=1) as wpool,
            tc.tile_pool(name="hpool", bufs=1) as hpool,
            tc.tile_pool(name="small", bufs=3) as small,
            tc.tile_pool(name="xtpool", bufs=1) as xtpool,
            tc.tile_pool(name="ypool", bufs=1) as ypool,
            tc.tile_pool(name="ps", bufs=8, space="PSUM") as ps,
        ):
            cph_sb = consts.tile([P, CH_COLS], bf16)
            cpf_sb = consts.tile([P, CF_COLS], f32)
            ident_sb = cph_sb[:, 0:P]
            w3_sb = cph_sb[:, P : P + KH]
            inv_sb = cpf_sb[:, 0:BL]
            b1_sb = cpf_sb[:, BL : BL + KH]
            b2_sb = cpf_sb[:, BL + KH : BL + 2 * KH]
            b3_sb = cpf_sb[0:1, BL + 2 * KH : BL + 2 * KH + 1]

            # Each dma_start is a ~610ns DIRECT2D instruction, serialized on
            # the Sync sequencer -- merge transfers aggressively and order
            # the few that remain by consumption time.
            mtn_sb = consts.tile([P, MTC], bf16)
            MB = 2 * KT * S  # one-hot cols for batches 0-1

            # hb0 split so pooling starts on the first 0.2 MB; everything
            # else arrives as single large transfers, all SBUF-resident
            hb0ks = [hpool.tile([P, H], bf16, tag=f"hb0k{k}", name=f"hb0k{k}")
                     for k in range(KT)]
            hb1 = hpool.tile([P, KT, H], bf16, tag="hb1")
            hbr = hpool.tile([P, BL - 2, KT, H], bf16, tag="hbr")
            w1_sb = wpool.tile([P, KH, H], bf16, tag="w1sb")
            w2_sb = wpool.tile([P, KH, H], bf16, tag="w2sb")

            nc.sync.dma_start(mtn_sb[:, 0:MB], mtn[:, 0:MB])
            for k in range(KT):
                nc.sync.dma_start(hb0ks[k], hid[0, k * P : (k + 1) * P, :])
            nc.sync.dma_start(cph_sb, cpack_h)
            nc.sync.dma_start(cpf_sb, cpack_f)
            nc.sync.dma_start(hb1, hid[1].rearrange("(k p) h -> p k h", p=P))
            nc.sync.dma_start(w1_sb, w1.rearrange("(k p) h -> p k h", p=P))
            nc.sync.dma_start(mtn_sb[:, MB:], mtn[:, MB:])
            for b in range(2, BL):
                nc.sync.dma_start(
                    hbr[:, b - 2], hid[b].rearrange("(k p) h -> p k h", p=P)
                )
            nc.sync.dma_start(w2_sb, w2.rearrange("(k p) h -> p k h", p=P))

            def hb_slice(b, k, lo, hi):
                if b == 0:
                    return hb0ks[k][:, lo:hi]
                if b == 1:
                    return hb1[:, k, lo:hi]
                return hbr[:, b - 2, k, lo:hi]

            w1ks = [w1_sb[:, k, :] for k in range(KH)]
            w2ks = [w2_sb[:, k, :] for k in range(KH)]

            xts = [xtpool.tile([P, R], bf16, tag=f"xt{k}", name=f"xt{k}") for k in range(KH)]
            y1s = [ypool.tile([P, R], bf16, tag=f"y1_{m}", name=f"y1_{m}") for m in range(KH)]
            y2s = [ypool.tile([P, R], bf16, tag=f"y2_{m}", name=f"y2_{m}") for m in range(KH)]
            logits = ypool.tile([1, R], f32, tag="logits")
            pred = ypool.tile([1, R], f32, tag="pred")

            C0 = 512          # pooling psum chunk 0: cols [0, 512)
            C1 = H - C0       # chunk 1: cols [512, 768)

            def pool(b):
                pp0 = ps.tile([P, C0], f32, tag="ps")
                pp1 = ps.tile([P, C1], f32, tag="ps")
                for k in range(KT):
                    # short MM first, long MM second: the next k's
                    # LDWEIGHTS fully hides under the 512-col stream
                    mt = mtn_sb[:, (b * KT + k) * S : (b * KT + k + 1) * S]
                    nc.tensor.matmul(
                        pp1, lhsT=mt, rhs=hb_slice(b, k, C0, H),
                        start=(k == 0), stop=(k == KT - 1),
                    )
                    nc.tensor.matmul(
                        pp0, lhsT=mt, rhs=hb_slice(b, k, 0, C0),
                        start=(k == 0), stop=(k == KT - 1),
                    )
                # evacuate psum * inv -> bf16 pooled. pp1 finishes its
                # accumulation one MM earlier than pp0, so evac its cols
                # first and transpose m4/m5 first: PE flows from the last
                # pool matmul straight into transposes with no DVE wait.
                pooled = small.tile([P, H], bf16, tag="pooled")
                ib = inv_sb[:, b : b + 1]
                nc.vector.tensor_tensor(
                    pooled[:, C0:H], pp1[:, 0:C1],
                    ib.to_broadcast((P, C1)), OP.mult,
                )
                nc.vector.tensor_tensor(
                    pooled[:, 0:P], pp0[:, 0:P], ib.to_broadcast((P, P)), OP.mult
                )
                nc.vector.tensor_tensor(
                    pooled[:, P:C0], pp0[:, P:C0],
                    ib.to_broadcast((P, C0 - P)), OP.mult,
                )
                for m in (4, 5, 0, 1, 2, 3):
                    trp = ps.tile([P, P], bf16, tag="ps")
                    nc.tensor.transpose(trp, pooled[:, m * P : (m + 1) * P], ident_sb)
                    nc.vector.tensor_copy(xts[m][:, b * S : (b + 1) * S], trp)

            def fc(wks, b_sb, xs, outs, rc, func):
                for m in range(KH):
                    pt = ps.tile([P, RC], f32, tag="ps")
                    for k in range(KH):
                        nc.tensor.matmul(
                            pt,
                            lhsT=wks[k][:, m * P : (m + 1) * P],
                            rhs=xs[k][:, rc * RC : (rc + 1) * RC],
                            start=(k == 0),
                            stop=(k == KH - 1),
                        )
                    nc.scalar.activation(
                        outs[m][:, rc * RC : (rc + 1) * RC],
                        pt,
                        func,
                        bias=b_sb[:, m : m + 1],
                    )

            def fc3mm(rc):
                ptl = ps.tile([1, RC], f32, tag="ps")
                for k in range(KH):
                    nc.tensor.matmul(
                        ptl,
                        lhsT=w3_sb[:, k : k + 1],
                        rhs=y2s[k][:, rc * RC : (rc + 1) * RC],
                        start=(k == 0),
                        stop=(k == KH - 1),
                    )
                nc.vector.tensor_copy(logits[:, rc * RC : (rc + 1) * RC], ptl)

            def sig(rc):
                nc.scalar.activation(
                    pred[:, rc * RC : (rc + 1) * RC],
                    logits[:, rc * RC : (rc + 1) * RC],
                    FT.Sigmoid,
                    bias=b3_sb,
                )

            G = FT.Gelu
            pool(0)
            pool(1)
            fc(w1ks, b1_sb, xts, y1s, 0, G)
            pool(2)
            pool(3)
            fc(w1ks, b1_sb, xts, y1s, 1, G)
            fc(w2ks, b2_sb, y1s, y2s, 0, G)
            fc3mm(0)
            pool(4)
            pool(5)
            fc(w1ks, b1_sb, xts, y1s, 2, G)
            fc(w2ks, b2_sb, y1s, y2s, 1, G)
            fc3mm(1)
            fc(w2ks, b2_sb, y1s, y2s, 2, G)
            fc3mm(2)
            pool(6)
            pool(7)
            # deferred sigmoids: one gelu->sigmoid table switch, hidden
            # under pool(6/7) matmuls on PE
            sig(0)
            sig(1)
            sig(2)
            fc(w1ks, b1_sb, xts, y1s, 3, G)
            fc(w2ks, b2_sb, y1s, y2s, 3, G)
            fc3mm(3)
            sig(3)
            nc.sync.dma_start(out.rearrange("b s -> (b s)"), pred[0:1, :])

    nc.compile()
    return nc


def _get_program():
    if "nc" not in _CACHE:
        _CACHE["nc"] = _build_program()
    return _CACHE["nc"]


def _cpack(sid_shard, b1, b2, b3, w3):
    """Per-core packed constants: bf16 (identity for PE transpose, w3) and
    f32 (inv = 1/max(count,1), biases). Plus the packed one-hot matrix."""
    oh = (sid_shard[:, :, None] == np.arange(S, dtype=np.int32)[None, None, :])
    counts = oh.sum(axis=1).astype(np.float32)          # [BL, S]
    inv = 1.0 / np.maximum(counts, 1.0)                 # [BL, S]
    mtn = np.ascontiguousarray(
        oh.reshape(BL, KT, P, S).transpose(2, 0, 1, 3).reshape(P, MTC)
    ).astype(BF16)
    ch = np.zeros((P, CH_COLS), dtype=BF16)
    ch[:, 0:P] = np.eye(P, dtype=np.float32)
    ch[:, P : P + KH] = np.asarray(w3, np.float32).reshape(KH, P).T
    cf = np.zeros((P, CF_COLS), dtype=np.float32)
    cf[:, 0:BL] = inv.T
    cf[:, BL : BL + KH] = np.asarray(b1, np.float32).reshape(KH, P).T
    cf[:, BL + KH : BL + 2 * KH] = np.asarray(b2, np.float32).reshape(KH, P).T
    cf[0, BL + 2 * KH] = np.float32(np.asarray(b3).reshape(-1)[0])
    return mtn, ch, cf


def make_in_maps(hidden, statements_ids, w1, b1, w2, b2, w3, b3):
    hidden = np.asarray(hidden, dtype=np.float32).astype(BF16)
    sid = np.asarray(statements_ids, dtype=np.int32)
    w1 = np.ascontiguousarray(np.asarray(w1, dtype=np.float32).astype(BF16))
    w2 = np.ascontiguousarray(np.asarray(w2, dtype=np.float32).astype(BF16))
    in_maps = []
    for c in range(N_CORES):
        mtn, ch, cf = _cpack(sid[c * BL : (c + 1) * BL], b1, b2, b3, w3)
        in_maps.append(
            {
                "hidden": np.ascontiguousarray(hidden[c * BL : (c + 1) * BL]),
                "mtn": mtn,
                "w1": w1,
                "w2": w2,
                "cpack_h": ch,
                "cpack_f": cf,
            }
        )
    return in_maps


def kernel(hidden, statements_ids, w1, b1, w2, b2, w3, b3, **kwargs):
    nc = _get_program()
    in_maps = make_in_maps(hidden, statements_ids, w1, b1, w2, b2, w3, b3)
    trace = bool(int(os.environ.get("KERNEL_TRACE", "0")))
    res = bass_utils.run_bass_kernel_spmd(
        nc, in_maps, core_ids=list(range(N_CORES)), trace=trace
    )
    _CACHE["last_results"] = res
    out = np.concatenate([res.results[c]["out"] for c in range(N_CORES)], axis=0)
    return out.astype(np.float32)
